# revision 3
# baseline (speedup 1.0000x reference)
"""MAMDense kernel for Trainium2 (8 NeuronCores, SPMD over row shards).

C[i,j] = max_k(x[i,k]*W[j,k]) + min_k(x[i,k]*W[j,k]) + bias[j]

v3 strategy (fp16 products, fp32 reduction; rel err ~1.6e-3):
  - Shard the flattened row dim M=2048 across 8 cores (256 rows each).
  - Per core, compute C^T [N, MC]: partitions = output cols j, free = rows i.
  - Per row i:
      * 6 diag tiles dg16[c] = diag(x[i, c*128:(c+1)*128]) in fp16
        (tensor_scalar of a fp16 identity by the per-partition x^T column).
      * PE: 36 matmuls p[j, k] = W[j,k]*x[i,k] into PSUM fp32
        (lhsT = W^T chunk [k,j] fp16 stationary, rhs = dg16[c]).
      * ScalarE copies the second k-half of p PSUM -> SBUF.
      * One custom DVE op (MAM_DUAL_MINMAX_ANT) streams the PSUM half and
        the SBUF half (2 products/cycle/lane) and computes
        running-max + running-min + bias with 4 in-pipe scan accumulators;
        a zero-stride out AP keeps only the final element, written directly
        to ct[b][:, i].  One DVE pass does BOTH the max and min chains.
  - DMA ct -> HBM; host transposes/concats shards.

The custom DVE op is registered at runtime into concourse.dve_ops (the
uop table is generated per-NEFF, no firmware change).  This walrus build
needs codegen_inst_isa_subclasses() run explicitly to fill InstISA bytes,
and accepts only ONE semaphore wait per instruction (post-pass splits
extra waits onto NoOps; the Tile drain is patched the same way).
"""

import os
import numpy as np

M_FULL, K, N, NCORES = 2048, 768, 768, 8
MC = M_FULL // NCORES
JB = N // 128
KC = K // 128
HALF = K // 2
FMAX = float(np.finfo(np.float32).max)

_STATE = {}
LAST_RUN_SECONDS = None

OP_NAME = "MAM_DUAL_MINMAX_ANT"


# --------------------------------------------------------------------------
# custom DVE op: dual min/max scan over two product streams, + bias
# --------------------------------------------------------------------------
def _register_mam_op():
    import concourse.dve_ops as dve_ops

    for o in dve_ops.OPS:
        if o.name == OP_NAME:
            return o

    from concourse.dve_spec import (
        Spec,
        Src0,
        Src1,
        C0,
        C1,
        AluOp,
        scan,
        maxx,
        minn,
        lower,
        _has_src1,
    )
    from concourse.dve_uop import DveOpSpec

    body = (
        maxx(scan(AluOp.MAX, Src0), scan(AluOp.MAX, Src1))
        + minn(scan(AluOp.MIN, Src0, init=C0), scan(AluOp.MIN, Src1, init=C0))
    ) + C1

    def ref(in0, in1, s0, s1, imm2):
        f32 = np.float32
        a0 = np.maximum.accumulate(in0.astype(f32), axis=-1)
        a1 = np.maximum.accumulate(in1.astype(f32), axis=-1)
        s0a = np.broadcast_to(np.asarray(s0, f32).reshape(-1, 1), in0.shape[:1] + (1,))
        m0 = np.minimum(np.minimum.accumulate(in0.astype(f32), axis=-1), s0a)
        m1 = np.minimum(np.minimum.accumulate(in1.astype(f32), axis=-1), s0a)
        s1a = np.broadcast_to(np.asarray(s1, f32).reshape(-1, 1), in0.shape[:1] + (1,))
        return (np.maximum(a0, a1) + np.minimum(m0, m1) + s1a).astype(f32)

    spec = Spec(body=body, reference=ref)
    row = dve_ops._CUSTOM_DVE_ROW_BASE + len(dve_ops.OPS)
    shas = {}
    for ver in ("v3", "v4"):
        try:
            uops = lower(spec, ver=ver)
            s = DveOpSpec(name=OP_NAME, opcode=row, uops=uops, rd1_en=_has_src1(spec))
            shas[ver] = s.sha(ver)
        except Exception:
            pass

    op = dve_ops.DveOp(OP_NAME, spec, subdim=False, uops_sha=shas)
    dve_ops.OPS.append(op)
    dve_ops.CUSTOM_DVE_SPECS[OP_NAME] = spec
    dve_ops._SUB_OPCODE_FOR_NAME[OP_NAME] = row
    return op


# --------------------------------------------------------------------------
# walrus single-sem-wait workarounds (carried over from the v1 baseline)
# --------------------------------------------------------------------------
def _patch_tile_drain(tile, mybir, ScopedClock, maxw=1):
    if getattr(tile.TileContext, "_mam_drain_patched", False):
        return

    def _pd(self, tick_clock, wait_clock):
        nc = self.nc
        drain_inst = nc.sync.drain()
        wait_clock.add_sem_waits(
            drain_inst.ins, ScopedClock({None: tick_clock.global_clock})
        )
        si = drain_inst.ins.sync_info
        waits = list(si.on_wait) if si is not None else []
        if len(waits) > maxw:
            si.on_wait = waits[:maxw]
            for i in range(maxw, len(waits), maxw):
                nop = nc.sync.nop(nofuse=True, hint="waitsplit")
                nop.ins.sync_info = mybir.SyncInfo(
                    on_wait=list(waits[i : i + maxw]), on_update=[]
                )
        nc.all_engine_barrier()
        popped = nc._tile_sem_poison_stack.pop()
        assert popped is self._sem_poison
        nc.clear_and_free_semaphores(list(self.sems.allocated().values()))
        nc.all_engine_barrier()

    tile.TileContext._drain_and_barrier = _pd
    tile.TileContext._mam_drain_patched = True


def _split_sem_waits(nc, mybir, maxw=1):
    n = 0
    for f in nc.m.functions:
        for blk in f.blocks:
            insts = blk.instructions
            i = 0
            while i < len(insts):
                inst = insts[i]
                si = inst.sync_info
                if si is not None and len(si.on_wait) > maxw:
                    waits = list(si.on_wait)
                    si.on_wait = waits[:maxw]
                    rest = waits[maxw:]
                    for j in range(0, len(rest), maxw):
                        n += 1
                        nop = mybir.InstNoOp(
                            name=f"I-wsplit-{n}-{inst.name}",
                            engine=inst.engine,
                            ins=[],
                            outs=[],
                            sync_info=mybir.SyncInfo(
                                on_wait=list(rest[j : j + maxw]), on_update=[]
                            ),
                        )
                        nc.register_instruction(nop)
                        insts.insert(i, nop)
                        i += 1
                i += 1
    return n


# --------------------------------------------------------------------------
# v3 builder
# --------------------------------------------------------------------------
def _build_nc_v3(loop_n=1, diag_engine="vector"):
    import contextlib
    import concourse.bass as bass
    import concourse.tile as tile
    import concourse.mybir as mybir
    from concourse.vector_clock import ScopedClock

    mam_op = _register_mam_op()
    _patch_tile_drain(tile, mybir, ScopedClock)

    F32 = mybir.dt.float32
    F16 = mybir.dt.float16

    nc = bass.Bass("TRN2", debug=False)
    wt16 = nc.dram_tensor("wt16", [K, N], F16, kind="ExternalInput")  # weight.T fp16
    xt32 = nc.dram_tensor("xt32", [K, MC], F32, kind="ExternalInput")  # x-shard^T fp32
    id16 = nc.dram_tensor("id16", [128, 128], F16, kind="ExternalInput")
    bias = nc.dram_tensor("bias_in", [N], F32, kind="ExternalInput")
    ct = nc.dram_tensor("ct", [N, MC], F32, kind="ExternalOutput")  # C^T shard

    with tile.TileContext(nc) as tc:
        loop_cm = tc.For_i(0, loop_n, 1) if loop_n > 1 else contextlib.nullcontext()
        with loop_cm, tc.tile_pool(name="singles", bufs=1) as singles, tc.tile_pool(
            name="dgpool", bufs=12
        ) as dgpool, tc.tile_pool(name="pspool", bufs=4, space="PSUM") as pspool, tc.tile_pool(
            name="sbpool", bufs=6
        ) as sbpool:
            # -------- setup --------
            w_sb = [
                [
                    singles.tile([128, 128], F16, tag=f"w{c}_{b}", name=f"w{c}_{b}")
                    for b in range(JB)
                ]
                for c in range(KC)
            ]
            for c in range(KC):
                for b in range(JB):
                    nc.sync.dma_start(
                        out=w_sb[c][b][:],
                        in_=wt16.ap()[c * 128 : (c + 1) * 128, b * 128 : (b + 1) * 128],
                    )
            xt_sb = [
                singles.tile([128, MC], F32, tag=f"xt{c}", name=f"xt{c}")
                for c in range(KC)
            ]
            for c in range(KC):
                nc.sync.dma_start(
                    out=xt_sb[c][:], in_=xt32.ap()[c * 128 : (c + 1) * 128, :]
                )
            id_sb = singles.tile([128, 128], F16, tag="id16")
            nc.sync.dma_start(out=id_sb[:], in_=id16.ap())
            bias_sb = singles.tile([128, JB], F32, tag="bias")
            nc.sync.dma_start(
                out=bias_sb[:], in_=bias.ap().rearrange("(b p) -> p b", p=128)
            )
            ct_sb = [
                singles.tile([128, MC], F32, tag=f"ct{b}", name=f"ct{b}")
                for b in range(JB)
            ]

            diag_eng = nc.gpsimd if diag_engine == "gpsimd" else nc.vector

            # -------- main loop over rows --------
            for i in range(MC):
                dgs = []
                for c in range(KC):
                    dg = dgpool.tile([128, 128], F16, tag=f"dg{c}")
                    diag_eng.tensor_scalar(
                        out=dg[:],
                        in0=id_sb[:],
                        scalar1=xt_sb[c][:, i : i + 1],
                        scalar2=None,
                        op0=mybir.AluOpType.mult,
                    )
                    dgs.append(dg)
                for b in range(JB):
                    p_ps = pspool.tile([128, K], F32, tag="pp")
                    for c in range(KC):
                        nc.tensor.matmul(
                            out=p_ps[:, c * 128 : (c + 1) * 128],
                            lhsT=w_sb[c][b][:],
                            rhs=dgs[c][:],
                            start=True,
                            stop=True,
                        )
                    p_sb = sbpool.tile([128, HALF], F32, tag="psb")
                    nc.scalar.copy(out=p_sb[:], in_=p_ps[:, HALF:])
                    base = ct_sb[b][:, i : i + 1]
                    out_z = bass.AP(
                        tensor=base.tensor,
                        offset=base.offset,
                        ap=[list(base.ap[0]), [0, HALF]],
                    )
                    nc.vector._custom_dve(
                        mam_op,
                        out=out_z,
                        in0=p_ps[:, :HALF],
                        in1=p_sb[:],
                        s0=FMAX,
                        s1=bias_sb[:, b : b + 1],
                    )
            # -------- writeback --------
            ct_re = ct.ap().rearrange("(b p) m -> b p m", p=128)
            for b in range(JB):
                nc.sync.dma_start(out=ct_re[b], in_=ct_sb[b][:])

    mybir.codegen_inst_isa_subclasses(nc)
    _split_sem_waits(nc, mybir)
    return nc


# --------------------------------------------------------------------------
# v1 builder (fallback, from the baseline)
# --------------------------------------------------------------------------
def _build_nc_v1(loop_n=1):
    import contextlib
    import concourse.bass as bass
    import concourse.tile as tile
    import concourse.mybir as mybir
    from concourse.vector_clock import ScopedClock

    _patch_tile_drain(tile, mybir, ScopedClock)

    DT = mybir.dt.float32
    nc = bass.Bass("TRN2", debug=False)
    xs = nc.dram_tensor("xs", [MC, K], DT, kind="ExternalInput")
    wt = nc.dram_tensor("wt", [K, N], DT, kind="ExternalInput")
    bias = nc.dram_tensor("bias_in", [N], DT, kind="ExternalInput")
    out = nc.dram_tensor("out", [MC, N], DT, kind="ExternalOutput")
    with tile.TileContext(nc) as tc:
        loop_cm = tc.For_i(0, loop_n, 1) if loop_n > 1 else contextlib.nullcontext()
        with loop_cm, tc.tile_pool(name="singles", bufs=1) as singles, tc.tile_pool(
            name="bpool", bufs=32
        ) as bpool:
            x_re = xs.ap().rearrange("(t p) k -> t p k", p=128)
            o_re = out.ap().rearrange("(t p) n -> t p n", p=128)
            ntiles = MC // 128
            x_sb, amax, amin = [], [], []
            for t in range(ntiles):
                xt = singles.tile([128, K], DT, tag=f"x{t}")
                nc.sync.dma_start(out=xt[:], in_=x_re[t])
                x_sb.append(xt)
                mx = singles.tile([128, N], DT, tag=f"amax{t}")
                mn = singles.tile([128, N], DT, tag=f"amin{t}")
                nc.vector.memset(mx[:], -FMAX)
                nc.vector.memset(mn[:], FMAX)
                amax.append(mx)
                amin.append(mn)
            biasb = singles.tile([128, N], DT, tag="biasb")
            bap = bias.ap()
            nc.sync.dma_start(
                out=biasb[:],
                in_=bass.AP(
                    tensor=bap.tensor, offset=bap.offset, ap=[[0, 128], list(bap.ap[0])]
                ),
            )
            for k in range(K):
                bt = bpool.tile([128, N], DT, tag="b")
                row = wt.ap()[k : k + 1, :]
                nc.sync.dma_start(
                    out=bt[:],
                    in_=bass.AP(
                        tensor=row.tensor,
                        offset=row.offset,
                        ap=[[0, 128], list(row.ap[1])],
                    ),
                )
                for t in range(ntiles):
                    nc.vector.scalar_tensor_tensor(
                        out=amax[t][:],
                        in0=bt[:],
                        scalar=x_sb[t][:, k : k + 1],
                        in1=amax[t][:],
                        op0=mybir.AluOpType.mult,
                        op1=mybir.AluOpType.max,
                    )
                    nc.vector.scalar_tensor_tensor(
                        out=amin[t][:],
                        in0=bt[:],
                        scalar=x_sb[t][:, k : k + 1],
                        in1=amin[t][:],
                        op0=mybir.AluOpType.mult,
                        op1=mybir.AluOpType.min,
                    )
            for t in range(ntiles):
                nc.vector.tensor_tensor(
                    out=amax[t][:],
                    in0=amax[t][:],
                    in1=amin[t][:],
                    op=mybir.AluOpType.add,
                )
                nc.vector.tensor_tensor(
                    out=amax[t][:],
                    in0=amax[t][:],
                    in1=biasb[:],
                    op=mybir.AluOpType.add,
                )
                nc.sync.dma_start(out=o_re[t], in_=amax[t][:])
    _split_sem_waits(nc, mybir)
    return nc


# --------------------------------------------------------------------------
# runner (bass2jax shard_map over 8 cores, from the baseline)
# --------------------------------------------------------------------------
def _make_runner(nc, n_cores=NCORES):
    import jax
    from jax.sharding import Mesh, PartitionSpec
    from jax.experimental.shard_map import shard_map
    import concourse.mybir as mybir
    from concourse import bass2jax

    bass2jax.install_neuronx_cc_hook()

    partition_name = nc.partition_id_tensor.name if nc.partition_id_tensor else None
    in_names, out_names, out_avals, zero_shapes = [], [], [], []
    for alloc in nc.m.functions[0].allocations:
        if not isinstance(alloc, mybir.MemoryLocationSet):
            continue
        name = alloc.memorylocations[0].name
        if alloc.kind == "ExternalInput":
            if name != partition_name:
                in_names.append(name)
        elif alloc.kind == "ExternalOutput":
            shape = tuple(alloc.tensor_shape)
            dtype = mybir.dt.np(alloc.dtype)
            out_names.append(name)
            out_avals.append(jax.core.ShapedArray(shape, dtype))
            zero_shapes.append((shape, dtype))
    n_params = len(in_names)
    n_outs = len(out_avals)
    in_names_all = list(in_names) + list(out_names)
    if partition_name is not None:
        in_names_all.append(partition_name)

    def _body(*args):
        operands = list(args)
        if partition_name is not None:
            operands.append(bass2jax.partition_id_tensor())
        outs = bass2jax._bass_exec_p.bind(
            *operands,
            out_avals=tuple(out_avals),
            in_names=tuple(in_names_all),
            out_names=tuple(out_names),
            lowering_input_output_aliases=(),
            sim_require_finite=True,
            sim_require_nnan=True,
            nc=nc,
        )
        return tuple(outs)

    devices = jax.devices()[:n_cores]
    mesh = Mesh(np.asarray(devices), ("core",))
    in_specs = (PartitionSpec("core"),) * (n_params + n_outs)
    out_specs = (PartitionSpec("core"),) * n_outs
    sharded = jax.jit(
        shard_map(
            _body, mesh=mesh, in_specs=in_specs, out_specs=out_specs, check_rep=False
        ),
        keep_unused=True,
    )

    def run(in_maps):
        global LAST_RUN_SECONDS
        import time as _time

        per_core = [[np.asarray(m[nm]) for nm in in_names] for m in in_maps]
        concat_in = [
            np.concatenate([per_core[c][i] for c in range(n_cores)], axis=0)
            for i in range(n_params)
        ]
        concat_zeros = [
            np.zeros((n_cores * s[0], *s[1:]), d) for (s, d) in zero_shapes
        ]
        t0 = _time.time()
        out_arrs = sharded(*concat_in, *concat_zeros)
        out_np = [np.asarray(a) for a in out_arrs]
        LAST_RUN_SECONDS = _time.time() - t0
        return [
            {
                nm: out_np[i].reshape(n_cores, *out_avals[i].shape)[c]
                for i, nm in enumerate(out_names)
            }
            for c in range(n_cores)
        ]

    run.sharded = sharded
    run.in_names = in_names
    run.zero_shapes = zero_shapes
    run.out_names = out_names
    run.out_avals = out_avals
    run.mesh = mesh
    return run


def _fallback_runner(nc):
    from concourse.bass_utils import run_bass_kernel_spmd

    def run(in_maps):
        res = run_bass_kernel_spmd(nc, in_maps, core_ids=list(range(NCORES)))
        return res.results

    return run


IMPL = os.environ.get("MAM_IMPL", "v3")
DIAG_ENGINE = os.environ.get("MAM_DIAG_ENGINE", "vector")


def _build(impl, loop_n=1):
    if impl == "v3":
        return _build_nc_v3(loop_n=loop_n, diag_engine=DIAG_ENGINE)
    return _build_nc_v1(loop_n=loop_n)


def _get_runner():
    if "runner" not in _STATE:
        impl = IMPL
        try:
            nc = _build(impl)
            _STATE["runner"] = _make_runner(nc)
            _STATE["impl"] = impl
        except Exception:
            nc = _build_nc_v1()
            _STATE["runner"] = _make_runner(nc)
            _STATE["impl"] = "v1"
    return _STATE["runner"], _STATE["impl"]


def _run_with_retry(run, in_maps, impl):
    try:
        return run(in_maps)
    except Exception:
        _STATE.pop("runner", None)
        nc = _build(impl)
        run2 = _fallback_runner(nc)
        return run2(in_maps)


def _in_maps_v3(xf, W, b):
    wt16 = np.ascontiguousarray(W.T.astype(np.float16))
    ident16 = np.eye(128, dtype=np.float16)
    b32 = np.ascontiguousarray(b.astype(np.float32))
    return [
        {
            "wt16": wt16,
            "xt32": np.ascontiguousarray(xf[c * MC : (c + 1) * MC].T.astype(np.float32)),
            "id16": ident16,
            "bias_in": b32,
        }
        for c in range(NCORES)
    ]


def _in_maps_v1(xf, W, b):
    wt = np.ascontiguousarray(W.T)
    b32 = np.ascontiguousarray(b.astype(np.float32))
    return [
        {"xs": xf[c * MC : (c + 1) * MC], "wt": wt, "bias_in": b32}
        for c in range(NCORES)
    ]


def kernel(x, weight, bias):
    x = np.ascontiguousarray(np.asarray(x, dtype=np.float32))
    W = np.ascontiguousarray(np.asarray(weight, dtype=np.float32))
    b = np.ascontiguousarray(np.asarray(bias, dtype=np.float32))
    run, impl = _get_runner()
    xf = x.reshape(-1, K)
    if impl == "v3":
        in_maps = _in_maps_v3(xf, W, b)
        outs = _run_with_retry(run, in_maps, impl)
        C = np.concatenate([o["ct"].T for o in outs], axis=0)
    else:
        in_maps = _in_maps_v1(xf, W, b)
        outs = _run_with_retry(run, in_maps, impl)
        C = np.concatenate([o["out"] for o in outs], axis=0)
    return np.ascontiguousarray(
        C.reshape(x.shape[:-1] + (W.shape[0],)), dtype=np.float32
    )


# revision 4
# speedup vs baseline: 3.0573x; 3.0573x over previous
"""MAMDense kernel for Trainium2 (8 NeuronCores, SPMD over row shards).

C[i,j] = max_k(x[i,k]*W[j,k]) + min_k(x[i,k]*W[j,k]) + bias[j]

v3 strategy (fp16 products, fp32 reduction; rel err ~1.6e-3):
  - Shard the flattened row dim M=2048 across 8 cores (256 rows each).
  - Per core, compute C^T [N, MC]: partitions = output cols j, free = rows i.
  - Per row i:
      * 6 diag tiles dg16[c] = diag(x[i, c*128:(c+1)*128]) in fp16
        (tensor_scalar of a fp16 identity by the per-partition x^T column).
      * PE: 36 matmuls p[j, k] = W[j,k]*x[i,k] into PSUM fp32
        (lhsT = W^T chunk [k,j] fp16 stationary, rhs = dg16[c]).
      * ScalarE copies the second k-half of p PSUM -> SBUF.
      * One custom DVE op (MAM_DUAL_MINMAX_ANT) streams the PSUM half and
        the SBUF half (2 products/cycle/lane) and computes
        running-max + running-min + bias with 4 in-pipe scan accumulators;
        a zero-stride out AP keeps only the final element, written directly
        to ct[b][:, i].  One DVE pass does BOTH the max and min chains.
  - DMA ct -> HBM; host transposes/concats shards.

The custom DVE op is registered at runtime into concourse.dve_ops (the
uop table is generated per-NEFF, no firmware change).  This walrus build
needs codegen_inst_isa_subclasses() run explicitly to fill InstISA bytes,
and accepts only ONE semaphore wait per instruction (post-pass splits
extra waits onto NoOps; the Tile drain is patched the same way).
"""

import os
import numpy as np

M_FULL, K, N, NCORES = 2048, 768, 768, 8
MC = M_FULL // NCORES
JB = N // 128
KC = K // 128
HALF = K // 2
FMAX = float(np.finfo(np.float32).max)

_STATE = {}
LAST_RUN_SECONDS = None

OP_NAME = "MAM_DUAL_MINMAX_ANT"


# --------------------------------------------------------------------------
# custom DVE op: dual min/max scan over two product streams, + bias
# --------------------------------------------------------------------------
def _register_mam_op():
    import concourse.dve_ops as dve_ops

    for o in dve_ops.OPS:
        if o.name == OP_NAME:
            return o

    from concourse.dve_spec import (
        Spec,
        Src0,
        Src1,
        C0,
        C1,
        AluOp,
        scan,
        maxx,
        minn,
        lower,
        _has_src1,
    )
    from concourse.dve_uop import DveOpSpec

    body = (
        maxx(scan(AluOp.MAX, Src0), scan(AluOp.MAX, Src1))
        + minn(scan(AluOp.MIN, Src0, init=C0), scan(AluOp.MIN, Src1, init=C0))
    ) + C1

    def ref(in0, in1, s0, s1, imm2):
        f32 = np.float32
        a0 = np.maximum.accumulate(in0.astype(f32), axis=-1)
        a1 = np.maximum.accumulate(in1.astype(f32), axis=-1)
        s0a = np.broadcast_to(np.asarray(s0, f32).reshape(-1, 1), in0.shape[:1] + (1,))
        m0 = np.minimum(np.minimum.accumulate(in0.astype(f32), axis=-1), s0a)
        m1 = np.minimum(np.minimum.accumulate(in1.astype(f32), axis=-1), s0a)
        s1a = np.broadcast_to(np.asarray(s1, f32).reshape(-1, 1), in0.shape[:1] + (1,))
        return (np.maximum(a0, a1) + np.minimum(m0, m1) + s1a).astype(f32)

    spec = Spec(body=body, reference=ref)
    row = dve_ops._CUSTOM_DVE_ROW_BASE + len(dve_ops.OPS)
    shas = {}
    for ver in ("v3", "v4"):
        try:
            uops = lower(spec, ver=ver)
            s = DveOpSpec(name=OP_NAME, opcode=row, uops=uops, rd1_en=_has_src1(spec))
            shas[ver] = s.sha(ver)
        except Exception:
            pass

    op = dve_ops.DveOp(OP_NAME, spec, subdim=False, uops_sha=shas)
    dve_ops.OPS.append(op)
    dve_ops.CUSTOM_DVE_SPECS[OP_NAME] = spec
    dve_ops._SUB_OPCODE_FOR_NAME[OP_NAME] = row
    return op


# --------------------------------------------------------------------------
# walrus single-sem-wait workarounds (carried over from the v1 baseline)
# --------------------------------------------------------------------------
def _patch_tile_drain(tile, mybir, ScopedClock, maxw=1):
    if getattr(tile.TileContext, "_mam_drain_patched", False):
        return

    def _pd(self, tick_clock, wait_clock):
        nc = self.nc
        drain_inst = nc.sync.drain()
        wait_clock.add_sem_waits(
            drain_inst.ins, ScopedClock({None: tick_clock.global_clock})
        )
        si = drain_inst.ins.sync_info
        waits = list(si.on_wait) if si is not None else []
        if len(waits) > maxw:
            si.on_wait = waits[:maxw]
            for i in range(maxw, len(waits), maxw):
                nop = nc.sync.nop(nofuse=True, hint="waitsplit")
                nop.ins.sync_info = mybir.SyncInfo(
                    on_wait=list(waits[i : i + maxw]), on_update=[]
                )
        nc.all_engine_barrier()
        popped = nc._tile_sem_poison_stack.pop()
        assert popped is self._sem_poison
        nc.clear_and_free_semaphores(list(self.sems.allocated().values()))
        nc.all_engine_barrier()

    tile.TileContext._drain_and_barrier = _pd
    tile.TileContext._mam_drain_patched = True


def _split_sem_waits(nc, mybir, maxw=1):
    n = 0
    for f in nc.m.functions:
        for blk in f.blocks:
            insts = blk.instructions
            i = 0
            while i < len(insts):
                inst = insts[i]
                si = inst.sync_info
                if si is not None and len(si.on_wait) > maxw:
                    waits = list(si.on_wait)
                    si.on_wait = waits[:maxw]
                    rest = waits[maxw:]
                    for j in range(0, len(rest), maxw):
                        n += 1
                        nop = mybir.InstNoOp(
                            name=f"I-wsplit-{n}-{inst.name}",
                            engine=inst.engine,
                            ins=[],
                            outs=[],
                            sync_info=mybir.SyncInfo(
                                on_wait=list(rest[j : j + maxw]), on_update=[]
                            ),
                        )
                        nc.register_instruction(nop)
                        insts.insert(i, nop)
                        i += 1
                i += 1
    return n


# --------------------------------------------------------------------------
# v3 builder
# --------------------------------------------------------------------------
def _build_nc_v3(loop_n=1, diag_engine="vector"):
    import contextlib
    import concourse.bass as bass
    import concourse.tile as tile
    import concourse.mybir as mybir
    from concourse.vector_clock import ScopedClock

    mam_op = _register_mam_op()
    _patch_tile_drain(tile, mybir, ScopedClock)

    F32 = mybir.dt.float32
    F16 = mybir.dt.float16

    nc = bass.Bass("TRN2", debug=False)
    wt16 = nc.dram_tensor("wt16", [K, N], F16, kind="ExternalInput")  # weight.T fp16
    xt32 = nc.dram_tensor("xt32", [K, MC], F32, kind="ExternalInput")  # x-shard^T fp32
    id16 = nc.dram_tensor("id16", [128, 128], F16, kind="ExternalInput")
    bias = nc.dram_tensor("bias_in", [N], F32, kind="ExternalInput")
    ct = nc.dram_tensor("ct", [N, MC], F32, kind="ExternalOutput")  # C^T shard

    with tile.TileContext(nc) as tc:
        loop_cm = tc.For_i(0, loop_n, 1) if loop_n > 1 else contextlib.nullcontext()
        with loop_cm, tc.tile_pool(name="singles", bufs=1) as singles, tc.tile_pool(
            name="dgpool", bufs=12
        ) as dgpool, tc.tile_pool(name="pspool", bufs=4, space="PSUM") as pspool, tc.tile_pool(
            name="sbpool", bufs=6
        ) as sbpool:
            # -------- setup --------
            w_sb = [
                [
                    singles.tile([128, 128], F16, tag=f"w{c}_{b}", name=f"w{c}_{b}")
                    for b in range(JB)
                ]
                for c in range(KC)
            ]
            for c in range(KC):
                for b in range(JB):
                    nc.sync.dma_start(
                        out=w_sb[c][b][:],
                        in_=wt16.ap()[c * 128 : (c + 1) * 128, b * 128 : (b + 1) * 128],
                    )
            xt_sb = [
                singles.tile([128, MC], F32, tag=f"xt{c}", name=f"xt{c}")
                for c in range(KC)
            ]
            for c in range(KC):
                nc.sync.dma_start(
                    out=xt_sb[c][:], in_=xt32.ap()[c * 128 : (c + 1) * 128, :]
                )
            id_sb = singles.tile([128, 128], F16, tag="id16")
            nc.sync.dma_start(out=id_sb[:], in_=id16.ap())
            bias_sb = singles.tile([128, JB], F32, tag="bias")
            nc.sync.dma_start(
                out=bias_sb[:], in_=bias.ap().rearrange("(b p) -> p b", p=128)
            )
            ct_sb = [
                singles.tile([128, MC], F32, tag=f"ct{b}", name=f"ct{b}")
                for b in range(JB)
            ]

            # how many of the 6 per-row diag builds go to ScalarE (rest on DVE)
            n_scal = int(os.environ.get("MAM_DIAG_SCAL", "3"))

            # -------- main loop over rows --------
            for i in range(MC):
                dgs = []
                for c in range(KC):
                    dg = dgpool.tile([128, 128], F16, tag=f"dg{c}")
                    if diag_engine == "gpsimd":
                        nc.gpsimd.tensor_scalar(
                            out=dg[:],
                            in0=id_sb[:],
                            scalar1=xt_sb[c][:, i : i + 1],
                            scalar2=None,
                            op0=mybir.AluOpType.mult,
                        )
                    elif c < n_scal:
                        nc.scalar.activation(
                            out=dg[:],
                            in_=id_sb[:],
                            func=mybir.ActivationFunctionType.Copy,
                            scale=xt_sb[c][:, i : i + 1],
                        )
                    else:
                        nc.vector.tensor_scalar(
                            out=dg[:],
                            in0=id_sb[:],
                            scalar1=xt_sb[c][:, i : i + 1],
                            scalar2=None,
                            op0=mybir.AluOpType.mult,
                        )
                    dgs.append(dg)
                for b in range(JB):
                    p_ps = pspool.tile([128, K], F32, tag="pp")
                    for c in range(KC):
                        nc.tensor.matmul(
                            out=p_ps[:, c * 128 : (c + 1) * 128],
                            lhsT=w_sb[c][b][:],
                            rhs=dgs[c][:],
                            start=True,
                            stop=True,
                        )
                    p_sb = sbpool.tile([128, HALF], F32, tag="psb")
                    nc.scalar.copy(out=p_sb[:], in_=p_ps[:, HALF:])
                    base = ct_sb[b][:, i : i + 1]
                    out_z = bass.AP(
                        tensor=base.tensor,
                        offset=base.offset,
                        ap=[list(base.ap[0]), [0, HALF]],
                    )
                    nc.vector._custom_dve(
                        mam_op,
                        out=out_z,
                        in0=p_ps[:, :HALF],
                        in1=p_sb[:],
                        s0=FMAX,
                        s1=bias_sb[:, b : b + 1],
                    )
            # -------- writeback --------
            ct_re = ct.ap().rearrange("(b p) m -> b p m", p=128)
            for b in range(JB):
                nc.sync.dma_start(out=ct_re[b], in_=ct_sb[b][:])

    mybir.codegen_inst_isa_subclasses(nc)
    _split_sem_waits(nc, mybir)
    return nc


# --------------------------------------------------------------------------
# v1 builder (fallback, from the baseline)
# --------------------------------------------------------------------------
def _build_nc_v1(loop_n=1):
    import contextlib
    import concourse.bass as bass
    import concourse.tile as tile
    import concourse.mybir as mybir
    from concourse.vector_clock import ScopedClock

    _patch_tile_drain(tile, mybir, ScopedClock)

    DT = mybir.dt.float32
    nc = bass.Bass("TRN2", debug=False)
    xs = nc.dram_tensor("xs", [MC, K], DT, kind="ExternalInput")
    wt = nc.dram_tensor("wt", [K, N], DT, kind="ExternalInput")
    bias = nc.dram_tensor("bias_in", [N], DT, kind="ExternalInput")
    out = nc.dram_tensor("out", [MC, N], DT, kind="ExternalOutput")
    with tile.TileContext(nc) as tc:
        loop_cm = tc.For_i(0, loop_n, 1) if loop_n > 1 else contextlib.nullcontext()
        with loop_cm, tc.tile_pool(name="singles", bufs=1) as singles, tc.tile_pool(
            name="bpool", bufs=32
        ) as bpool:
            x_re = xs.ap().rearrange("(t p) k -> t p k", p=128)
            o_re = out.ap().rearrange("(t p) n -> t p n", p=128)
            ntiles = MC // 128
            x_sb, amax, amin = [], [], []
            for t in range(ntiles):
                xt = singles.tile([128, K], DT, tag=f"x{t}")
                nc.sync.dma_start(out=xt[:], in_=x_re[t])
                x_sb.append(xt)
                mx = singles.tile([128, N], DT, tag=f"amax{t}")
                mn = singles.tile([128, N], DT, tag=f"amin{t}")
                nc.vector.memset(mx[:], -FMAX)
                nc.vector.memset(mn[:], FMAX)
                amax.append(mx)
                amin.append(mn)
            biasb = singles.tile([128, N], DT, tag="biasb")
            bap = bias.ap()
            nc.sync.dma_start(
                out=biasb[:],
                in_=bass.AP(
                    tensor=bap.tensor, offset=bap.offset, ap=[[0, 128], list(bap.ap[0])]
                ),
            )
            for k in range(K):
                bt = bpool.tile([128, N], DT, tag="b")
                row = wt.ap()[k : k + 1, :]
                nc.sync.dma_start(
                    out=bt[:],
                    in_=bass.AP(
                        tensor=row.tensor,
                        offset=row.offset,
                        ap=[[0, 128], list(row.ap[1])],
                    ),
                )
                for t in range(ntiles):
                    nc.vector.scalar_tensor_tensor(
                        out=amax[t][:],
                        in0=bt[:],
                        scalar=x_sb[t][:, k : k + 1],
                        in1=amax[t][:],
                        op0=mybir.AluOpType.mult,
                        op1=mybir.AluOpType.max,
                    )
                    nc.vector.scalar_tensor_tensor(
                        out=amin[t][:],
                        in0=bt[:],
                        scalar=x_sb[t][:, k : k + 1],
                        in1=amin[t][:],
                        op0=mybir.AluOpType.mult,
                        op1=mybir.AluOpType.min,
                    )
            for t in range(ntiles):
                nc.vector.tensor_tensor(
                    out=amax[t][:],
                    in0=amax[t][:],
                    in1=amin[t][:],
                    op=mybir.AluOpType.add,
                )
                nc.vector.tensor_tensor(
                    out=amax[t][:],
                    in0=amax[t][:],
                    in1=biasb[:],
                    op=mybir.AluOpType.add,
                )
                nc.sync.dma_start(out=o_re[t], in_=amax[t][:])
    _split_sem_waits(nc, mybir)
    return nc


# --------------------------------------------------------------------------
# runner (bass2jax shard_map over 8 cores, from the baseline)
# --------------------------------------------------------------------------
def _make_runner(nc, n_cores=NCORES):
    import jax
    from jax.sharding import Mesh, PartitionSpec
    from jax.experimental.shard_map import shard_map
    import concourse.mybir as mybir
    from concourse import bass2jax

    bass2jax.install_neuronx_cc_hook()

    partition_name = nc.partition_id_tensor.name if nc.partition_id_tensor else None
    in_names, out_names, out_avals, zero_shapes = [], [], [], []
    for alloc in nc.m.functions[0].allocations:
        if not isinstance(alloc, mybir.MemoryLocationSet):
            continue
        name = alloc.memorylocations[0].name
        if alloc.kind == "ExternalInput":
            if name != partition_name:
                in_names.append(name)
        elif alloc.kind == "ExternalOutput":
            shape = tuple(alloc.tensor_shape)
            dtype = mybir.dt.np(alloc.dtype)
            out_names.append(name)
            out_avals.append(jax.core.ShapedArray(shape, dtype))
            zero_shapes.append((shape, dtype))
    n_params = len(in_names)
    n_outs = len(out_avals)
    in_names_all = list(in_names) + list(out_names)
    if partition_name is not None:
        in_names_all.append(partition_name)

    def _body(*args):
        operands = list(args)
        if partition_name is not None:
            operands.append(bass2jax.partition_id_tensor())
        outs = bass2jax._bass_exec_p.bind(
            *operands,
            out_avals=tuple(out_avals),
            in_names=tuple(in_names_all),
            out_names=tuple(out_names),
            lowering_input_output_aliases=(),
            sim_require_finite=True,
            sim_require_nnan=True,
            nc=nc,
        )
        return tuple(outs)

    devices = jax.devices()[:n_cores]
    mesh = Mesh(np.asarray(devices), ("core",))
    in_specs = (PartitionSpec("core"),) * (n_params + n_outs)
    out_specs = (PartitionSpec("core"),) * n_outs
    sharded = jax.jit(
        shard_map(
            _body, mesh=mesh, in_specs=in_specs, out_specs=out_specs, check_rep=False
        ),
        keep_unused=True,
    )

    def run(in_maps):
        global LAST_RUN_SECONDS
        import time as _time

        per_core = [[np.asarray(m[nm]) for nm in in_names] for m in in_maps]
        concat_in = [
            np.concatenate([per_core[c][i] for c in range(n_cores)], axis=0)
            for i in range(n_params)
        ]
        concat_zeros = [
            np.zeros((n_cores * s[0], *s[1:]), d) for (s, d) in zero_shapes
        ]
        t0 = _time.time()
        out_arrs = sharded(*concat_in, *concat_zeros)
        out_np = [np.asarray(a) for a in out_arrs]
        LAST_RUN_SECONDS = _time.time() - t0
        return [
            {
                nm: out_np[i].reshape(n_cores, *out_avals[i].shape)[c]
                for i, nm in enumerate(out_names)
            }
            for c in range(n_cores)
        ]

    run.sharded = sharded
    run.in_names = in_names
    run.zero_shapes = zero_shapes
    run.out_names = out_names
    run.out_avals = out_avals
    run.mesh = mesh
    return run


def _fallback_runner(nc):
    from concourse.bass_utils import run_bass_kernel_spmd

    def run(in_maps):
        res = run_bass_kernel_spmd(nc, in_maps, core_ids=list(range(NCORES)))
        return res.results

    return run


IMPL = os.environ.get("MAM_IMPL", "v3")
DIAG_ENGINE = os.environ.get("MAM_DIAG_ENGINE", "vector")


def _build(impl, loop_n=1):
    if impl == "v3":
        return _build_nc_v3(loop_n=loop_n, diag_engine=DIAG_ENGINE)
    return _build_nc_v1(loop_n=loop_n)


def _get_runner():
    if "runner" not in _STATE:
        impl = IMPL
        try:
            nc = _build(impl)
            _STATE["runner"] = _make_runner(nc)
            _STATE["impl"] = impl
        except Exception:
            nc = _build_nc_v1()
            _STATE["runner"] = _make_runner(nc)
            _STATE["impl"] = "v1"
    return _STATE["runner"], _STATE["impl"]


def _run_with_retry(run, in_maps, impl):
    try:
        return run(in_maps)
    except Exception:
        _STATE.pop("runner", None)
        nc = _build(impl)
        run2 = _fallback_runner(nc)
        return run2(in_maps)


def _in_maps_v3(xf, W, b):
    wt16 = np.ascontiguousarray(W.T.astype(np.float16))
    ident16 = np.eye(128, dtype=np.float16)
    b32 = np.ascontiguousarray(b.astype(np.float32))
    return [
        {
            "wt16": wt16,
            "xt32": np.ascontiguousarray(xf[c * MC : (c + 1) * MC].T.astype(np.float32)),
            "id16": ident16,
            "bias_in": b32,
        }
        for c in range(NCORES)
    ]


def _in_maps_v1(xf, W, b):
    wt = np.ascontiguousarray(W.T)
    b32 = np.ascontiguousarray(b.astype(np.float32))
    return [
        {"xs": xf[c * MC : (c + 1) * MC], "wt": wt, "bias_in": b32}
        for c in range(NCORES)
    ]


def kernel(x, weight, bias):
    x = np.ascontiguousarray(np.asarray(x, dtype=np.float32))
    W = np.ascontiguousarray(np.asarray(weight, dtype=np.float32))
    b = np.ascontiguousarray(np.asarray(bias, dtype=np.float32))
    run, impl = _get_runner()
    xf = x.reshape(-1, K)
    if impl == "v3":
        in_maps = _in_maps_v3(xf, W, b)
        outs = _run_with_retry(run, in_maps, impl)
        C = np.concatenate([o["ct"].T for o in outs], axis=0)
    else:
        in_maps = _in_maps_v1(xf, W, b)
        outs = _run_with_retry(run, in_maps, impl)
        C = np.concatenate([o["out"] for o in outs], axis=0)
    return np.ascontiguousarray(
        C.reshape(x.shape[:-1] + (W.shape[0],)), dtype=np.float32
    )
